# revision 7
# baseline (speedup 1.0000x reference)
"""Trainium2 Bass kernel for causal GQA attention (nn_CausalAttention).

Full-input contract: kernel(**inputs) takes the complete unsharded inputs and
returns the full [B, S, H] output. Internally shards across 8 NeuronCores as
(batch b in {0,1}) x (head-group g in {0..3}); each core computes 8 query heads
/ 2 KV heads for one batch and a row-parallel partial o_proj; the host sums the
4 partials per batch.

v2 design notes (vs the 310us baseline):
  - RoPE rotate_half is done with cross-partition-offset DVE adds (operand
    base partition != dest base partition) against a host-pre-shifted sin
    table; no SBUF->SBUF DMAs, no ACT staging copy (DVE reads PSUM directly).
  - Head-pair packing: a scores tile is (1 key tile) x (2 heads) row-packed on
    PE tiles (0,0)/(64,0); qT stores head 2m on partitions 0-63 and head 2m+1
    on 64-127 of slot m, so no qT duplication is needed. kT keeps both kv
    heads duplicated across halves (one DVE copy per chunk).
  - exp runs once per (slot, key tile) over [128, 2, span] (both heads share
    the same causal narrowing), halving ACT instruction count on diag tiles;
    the mask multiply is one broadcast TT per diag tile.
  - hidT / weights are host-relayouted to [128, k, .] so all loads are a few
    big DMAs (4 per hid chunk, 8 preload) instead of ~120 row-tile DMAs.
  - output DRAM tensor is bf16, staged per 128-row block and stored with one
    DMA per block (4/chunk); the host upcasts and sums partials in f32.
  - 1/l normalization multiplies ps_o[0:64] by ps_o[64:128] (the K=1 matmul
    broadcast) directly from PSUM - no bc staging copy.
"""

import math
import sys

import numpy as np

try:
    import concourse.bass as _probe  # noqa: F401
except ImportError:
    sys.path.insert(0, "/opt/trn_rl_repo")

import ml_dtypes

BF16 = ml_dtypes.bfloat16

# problem config (hardcoded per contract)
B, S, H = 2, 2048, 2048
NUM_HEADS, NUM_KV_HEADS, D = 32, 8, 64
NCORES = 8
GROUPS = 4                    # head-groups = cores per batch
QH = NUM_HEADS // GROUPS      # 8 q heads per core
KVH = NUM_KV_HEADS // GROUPS  # 2 kv heads per core
DQ = QH * D                   # 512
DKV = KVH * D                 # 128
KT = H // 128                 # 16 contraction tiles over hidden dim

P = 128
HD = D // 2
EXP_BIAS = -4.0

LAST_RESULTS = None
_NC_CACHE = {}


def make_cosr(cos_b, dtype=BF16):
    """cos tiled to [128, S]: row p = 64a + q holds cos[t, q]."""
    return np.ascontiguousarray(np.tile(cos_b.T, (2, 1))).astype(dtype)


def make_sinr(sin_b, dtype=BF16):
    """sin tiled to [128, S] with rotate_half's sign folded in: row p holds
    sign(partner(p)) * sin[t, d(p)] so that the shifted-output product
    prod_sh[p^32] = raw[p] * sinr[p] lands with the right sign."""
    sgn = np.where(np.arange(P) % D < HD, -1.0, 1.0).astype(np.float32)
    # sign belongs to the DESTINATION row p^32; fold sgn[p^32] into row p
    sgn_pre = sgn.reshape(2, 2, HD)[:, ::-1].reshape(P)
    return np.ascontiguousarray(
        np.tile(sin_b.T, (2, 1)) * sgn_pre[:, None]
    ).astype(dtype)


def make_diag_masks(tqc, dtype=np.float32):
    """masks[i, m, j] = 1 if key-row i of diag tile m is visible to query j."""
    ndiag = tqc // P
    masks = np.zeros((P, ndiag, tqc), dtype=np.float32)
    i = np.arange(P)[:, None]
    j = np.arange(tqc)[None, :]
    for m in range(ndiag):
        masks[:, m, :] = (i + P * m <= j).astype(np.float32)
    return masks.astype(dtype)


def build_attention_nc(S=S, H=H, TQC=512, n_repeat=1):
    """Build the single-core SPMD Bass program. TQC = query-chunk width."""
    import concourse.bass as bass  # noqa: F401
    import concourse.mybir as mybir
    import concourse.tile as tile
    from concourse import bacc
    from contextlib import ExitStack

    bf = mybir.dt.bfloat16
    f32 = mybir.dt.float32
    NTQ = S // TQC       # query chunks
    NTK = S // P         # key tiles
    GRP = TQC // P       # tiles per chunk == number of diag masks == slots
    NQO = DQ // P        # q slots (4); also yT column blocks
    NHO = H // TQC       # o_proj output chunks (4)
    scale = 1.0 / math.sqrt(D)

    nc = bacc.Bacc()
    hidT = nc.declare_dram_parameter("hidT", [P, KT, S], bf, isOutput=False)
    wqT = nc.declare_dram_parameter("wqT", [P, KT, DQ], bf, isOutput=False)
    wkT = nc.declare_dram_parameter("wkT", [P, KT, DKV], bf, isOutput=False)
    wvT = nc.declare_dram_parameter("wvT", [P, KT, DKV], bf, isOutput=False)
    woT = nc.declare_dram_parameter("woT", [P, NQO, H], bf, isOutput=False)
    cosr = nc.declare_dram_parameter("cosr", [P, S], bf, isOutput=False)
    sinr = nc.declare_dram_parameter("sinr", [P, S], bf, isOutput=False)
    masks = nc.declare_dram_parameter("masks", [P, GRP, TQC], bf, isOutput=False)
    out = nc.declare_dram_parameter("out", [S, H], bf, isOutput=True)

    MUL = mybir.AluOpType.mult
    ADD = mybir.AluOpType.add
    EXP = mybir.ActivationFunctionType.Exp

    with ExitStack() as ctx:
        tc = ctx.enter_context(tile.TileContext(nc))
        const = ctx.enter_context(tc.tile_pool(name="const", bufs=1))
        hidp = ctx.enter_context(tc.tile_pool(name="hidp", bufs=2))
        work = ctx.enter_context(tc.tile_pool(name="work", bufs=4))
        stage = ctx.enter_context(tc.tile_pool(name="stage", bufs=2))
        ppool = ctx.enter_context(tc.tile_pool(name="psmall", bufs=2, space="PSUM"))
        pbig = ctx.enter_context(tc.tile_pool(name="pbig", bufs=2, space="PSUM"))
        po = ctx.enter_context(tc.tile_pool(name="po", bufs=1, space="PSUM"))

        wq_sb = const.tile([P, KT, DQ], bf)
        wk_sb = const.tile([P, KT, DKV], bf)
        wv_sb = const.tile([P, KT, DKV], bf)
        wo_sb = const.tile([P, NQO, H], bf)
        cos_sb = const.tile([P, S], bf)
        sin_sb = const.tile([P, S], bf)
        mask_sb = const.tile([P, GRP, TQC], bf)
        # qT: slot m holds head 2m on partitions 0-63 and head 2m+1 on 64-127.
        # kT: kv heads on partitions 0-63, duplicated to 64-127 for row-packed
        # QK pairs (both heads of a pair read the same kv head).
        qT_sb = const.tile([P, NQO, S], bf)
        kT_sb = const.tile([P, KVH, S], bf)
        vA_sb = [const.tile([P, NTK, 65], bf, name=f"vA{k}") for k in range(KVH)]
        yT_sb = const.tile([P, NQO, S], bf)

        # --- preload (big DMAs; gpsimd/SWDGE for weights so the sync queue
        # stays free for hid chunks and output stores) ---
        nc.gpsimd.dma_start(wk_sb[:], wkT[:])
        nc.gpsimd.dma_start(wv_sb[:], wvT[:])
        nc.gpsimd.dma_start(cos_sb[:], cosr[:])
        nc.gpsimd.dma_start(sin_sb[:], sinr[:])
        nc.gpsimd.dma_start(mask_sb[:], masks[:])
        for f in range(NQO):
            nc.gpsimd.dma_start(wo_sb[:, f, :], woT[:, f, :])
        for k in range(KVH):
            nc.vector.memset(vA_sb[k][:, :, 64:65], 1.0)
        exp_bias_sb = const.tile([P, 1], f32)
        nc.vector.memset(exp_bias_sb[:], EXP_BIAS)
        ones_col = const.tile([1, 64], bf)
        nc.vector.memset(ones_col[:], 1.0)

        hid_chs = {}

        def emit_hid_load(c):
            tq = slice(c * TQC, (c + 1) * TQC)
            hid_ch = hidp.tile([P, KT, TQC], bf, tag="hid_ch")
            for g4 in range(0, KT, 4):
                nc.sync.dma_start(
                    hid_ch[:, g4:g4 + 4, :], hidT[:, g4:g4 + 4, tq]
                )
            hid_chs[c] = hid_ch

        def rope_project(ps, dests, tq_sl):
            """ps: [P, TQC] PSUM with 2 heads of projected values (transposed).
            dests: two [64, TQC] APs (one per 64-row head block of ps).
            rotate_half is a partition-shifted OUTPUT on the sin product (the
            BIR verifier requires equal input base partitions but lets the
            output land elsewhere); the rotate sign is folded into sinr."""
            t_cos = work.tile([P, TQC], bf, tag="rope_cos")
            nc.vector.tensor_tensor(t_cos, ps, cos_sb[:, tq_sl], MUL)
            prod = work.tile([P, TQC], bf, tag="rope_sin")
            for o in (0, D):
                nc.vector.tensor_tensor(
                    prod[o:o + HD, :],
                    ps[o + HD:o + D, :],
                    sin_sb[o + HD:o + D, tq_sl],
                    MUL,
                )
                nc.vector.tensor_tensor(
                    prod[o + HD:o + D, :],
                    ps[o:o + HD, :],
                    sin_sb[o:o + HD, tq_sl],
                    MUL,
                )
            for hb, dest in enumerate(dests):
                o = hb * D
                nc.vector.tensor_tensor(
                    dest, t_cos[o:o + D, :], prod[o:o + D, :], ADD
                )

        def emit_k_proj(c):
            tq = slice(c * TQC, (c + 1) * TQC)
            ps_k = ppool.tile([P, TQC], f32, tag="ps_proj")
            for ki in range(KT):
                nc.tensor.matmul(
                    ps_k,
                    wk_sb[:, ki, :],
                    hid_chs[c][:, ki, :],
                    start=(ki == 0),
                    stop=(ki == KT - 1),
                )
            rope_project(ps_k, [kT_sb[0:D, 0, tq], kT_sb[0:D, 1, tq]], tq)
            nc.vector.tensor_copy(kT_sb[D:2 * D, :, tq], kT_sb[0:D, :, tq])

        def emit_v_proj(c, tt):
            tl = tt - c * GRP
            ps_v = ppool.tile([P, TQC], f32, tag="ps_proj")
            for ki in range(KT):
                nc.tensor.matmul(
                    ps_v[:, 0:DKV],
                    hid_chs[c][:, ki, tl * P:(tl + 1) * P],
                    wv_sb[:, ki, :],
                    start=(ki == 0),
                    stop=(ki == KT - 1),
                )
            for k in range(KVH):
                nc.vector.tensor_copy(
                    vA_sb[k][:, tt, 0:64], ps_v[:, k * D:(k + 1) * D]
                )

        def emit_q_proj(c, oo):
            tq = slice(c * TQC, (c + 1) * TQC)
            ps_q = ppool.tile([P, TQC], f32, tag="ps_proj")
            for ki in range(KT):
                nc.tensor.matmul(
                    ps_q,
                    wq_sb[:, ki, oo * P:(oo + 1) * P],
                    hid_chs[c][:, ki, :],
                    start=(ki == 0),
                    stop=(ki == KT - 1),
                )
            rope_project(
                ps_q, [qT_sb[0:D, oo, tq], qT_sb[D:2 * D, oo, tq]], tq
            )

        ostage = {}

        def emit_oproj_unit(c, tt, oc):
            ps_out = ppool.tile([P, TQC], f32, tag="ps_proj")
            for f in range(NQO):
                nc.tensor.matmul(
                    ps_out,
                    yT_sb[:, f, tt * P:(tt + 1) * P],
                    wo_sb[:, f, oc * TQC:(oc + 1) * TQC],
                    start=(f == 0),
                    stop=(f == NQO - 1),
                )
            if oc == 0:
                ostage[tt] = stage.tile(
                    [P, NHO, TQC], bf, tag="ostage", name="ostage"
                )
            st = ostage[tt]
            if (tt + oc) % 2 == 0:
                nc.vector.tensor_copy(st[:, oc, :], ps_out[:])
            else:
                nc.scalar.copy(st[:, oc, :], ps_out[:])
            if oc == NHO - 1:
                nc.sync.dma_start(out[tt * P:(tt + 1) * P, :], st[:])
                del ostage[tt]

        def run_main_loop():
            for c in range(NTQ):
                _run_chunk(c)
            for tt in range((NTQ - 1) * GRP, NTQ * GRP):
                for oc in range(NHO):
                    emit_oproj_unit(NTQ - 1, tt, oc)

        def _run_chunk(c):
            tq_sl = slice(c * TQC, (c + 1) * TQC)
            if c + 1 < NTQ:
                emit_hid_load(c + 1)
            fillers = []
            if c + 1 < NTQ:
                fillers.append(lambda cc=c + 1: emit_k_proj(cc))
                for tt in range((c + 1) * GRP, (c + 2) * GRP):
                    fillers.append(lambda cc=c + 1, t=tt: emit_v_proj(cc, t))
                for oo in range(NQO):
                    fillers.append(lambda cc=c + 1, o=oo: emit_q_proj(cc, o))
            if c >= 1:
                for tt in range((c - 1) * GRP, c * GRP):
                    for oc in range(NHO):
                        fillers.append(
                            lambda cc=c - 1, t=tt, o=oc: emit_oproj_unit(cc, t, o)
                        )
            popped = 0

            ntk = (c + 1) * GRP
            for m in range(GRP):  # head-pair slots
                kv = m // 2
                ps_o0 = po.tile([P, TQC], f32, tag="ps_o0")
                ps_o1 = po.tile([P, TQC], f32, tag="ps_o1")
                pos = (ps_o0, ps_o1)
                for t in range(ntk):
                    md = t - (ntk - GRP)
                    lo = md * P if md >= 0 else 0
                    ps_s = pbig.tile([P, 2, TQC], f32, tag="ps_s")
                    for j in range(2):
                        pb = D * j
                        nc.tensor.matmul(
                            ps_s[:, j, lo:],
                            kT_sb[pb:pb + D, kv, t * P:(t + 1) * P],
                            qT_sb[pb:pb + D, m, c * TQC + lo:(c + 1) * TQC],
                            start=True,
                            stop=True,
                        )
                    p_sb = work.tile([P, 2, TQC], bf, tag="p_sb")
                    nc.scalar.activation(
                        p_sb[:, :, lo:],
                        ps_s[:, :, lo:],
                        EXP,
                        bias=exp_bias_sb[:],
                        scale=scale,
                    )
                    if md >= 0:
                        mop = mask_sb[:, md, lo:].unsqueeze(1).broadcast_to(
                            (P, 2, TQC - lo)
                        )
                        nc.vector.tensor_tensor(
                            p_sb[:, :, lo:], p_sb[:, :, lo:], mop, MUL
                        )
                    for j in range(2):
                        nc.tensor.matmul(
                            pos[j][0:65, lo:],
                            vA_sb[kv][:, t, :],
                            p_sb[:, j, lo:],
                            start=(t == 0),
                            stop=(t == ntk - 1),
                        )
                for j in range(2):
                    ps_o = pos[j]
                    rec = work.tile([1, TQC], bf, tag="rec")
                    with nc.allow_low_precision(
                        reason="1/l broadcast via bf16 matmul; ~0.2% uniform scale noise"
                    ):
                        nc.vector.reciprocal(rec, ps_o[64:65, :])
                    nc.tensor.matmul(
                        ps_o[64:128, :], ones_col[:], rec[:], start=True, stop=True
                    )
                    # DVE may read only one operand from PSUM: stage the
                    # broadcast 1/l rows through SBUF
                    bc = work.tile([64, TQC], f32, tag="bc")
                    nc.vector.tensor_copy(bc[:], ps_o[64:128, :])
                    nc.vector.tensor_tensor(
                        yT_sb[D * j:D * (j + 1), m, tq_sl],
                        ps_o[0:64, :],
                        bc[:],
                        MUL,
                    )
                want = len(fillers) * (m + 1) // GRP
                while popped < want:
                    fillers[popped]()
                    popped += 1

        first = True
        for _rep in range(n_repeat):
            if first:
                emit_hid_load(0)
                # wq after hid0 on the sync queue: k/v proj need hid first
                for g4 in range(0, KT, 4):
                    nc.sync.dma_start(
                        wq_sb[:, g4:g4 + 4, :], wqT[:, g4:g4 + 4, :]
                    )
                first = False
            else:
                emit_hid_load(0)
            emit_k_proj(0)
            for tt in range(GRP):
                emit_v_proj(0, tt)
            for oo in range(NQO):
                emit_q_proj(0, oo)
            run_main_loop()

    nc.compile()
    return nc


def _get_nc():
    key = (S, H)
    if key not in _NC_CACHE:
        _NC_CACHE[key] = build_attention_nc()
    return _NC_CACHE[key]


def _prep_core_inputs(hidden_states, cos, sin, Wq, Wk, Wv, Wo):
    """Build the 8 per-core input maps (core index = b * GROUPS + g)."""
    TQC = 512
    masks_np = make_diag_masks(TQC, BF16)

    def relay(mat, k):
        """[k*128, n] -> [128, k, n] so row p, slice ki holds mat[ki*128+p]."""
        n = mat.shape[1]
        return np.ascontiguousarray(
            mat.reshape(k, P, n).transpose(1, 0, 2)
        ).astype(BF16)

    in_maps = []
    per_batch = {}
    for b in range(B):
        hidT = relay(np.ascontiguousarray(hidden_states[b].T), KT)
        cosr = make_cosr(cos[b])
        sinr = make_sinr(sin[b])
        per_batch[b] = (hidT, cosr, sinr)
    wq_g = [relay(np.ascontiguousarray(Wq[DQ * g:DQ * (g + 1), :].T), KT)
            for g in range(GROUPS)]
    wk_g = [relay(np.ascontiguousarray(Wk[DKV * g:DKV * (g + 1), :].T), KT)
            for g in range(GROUPS)]
    wv_g = [relay(np.ascontiguousarray(Wv[DKV * g:DKV * (g + 1), :].T), KT)
            for g in range(GROUPS)]
    wo_g = [relay(np.ascontiguousarray(Wo[:, DQ * g:DQ * (g + 1)].T), DQ // P)
            for g in range(GROUPS)]
    for b in range(B):
        hidT, cosr, sinr = per_batch[b]
        for g in range(GROUPS):
            in_maps.append({
                "hidT": hidT,
                "wqT": wq_g[g],
                "wkT": wk_g[g],
                "wvT": wv_g[g],
                "woT": wo_g[g],
                "cosr": cosr,
                "sinr": sinr,
                "masks": masks_np,
            })
    return in_maps


def kernel(hidden_states, cos, sin, Wq, Wk, Wv, Wo):
    global LAST_RESULTS
    from concourse.bass_utils import run_bass_kernel_spmd

    hidden_states = np.asarray(hidden_states, dtype=np.float32)
    cos = np.asarray(cos, dtype=np.float32)
    sin = np.asarray(sin, dtype=np.float32)
    Wq = np.asarray(Wq, dtype=np.float32)
    Wk = np.asarray(Wk, dtype=np.float32)
    Wv = np.asarray(Wv, dtype=np.float32)
    Wo = np.asarray(Wo, dtype=np.float32)
    assert hidden_states.shape == (B, S, H)

    nc = _get_nc()
    in_maps = _prep_core_inputs(hidden_states, cos, sin, Wq, Wk, Wv, Wo)
    res = run_bass_kernel_spmd(nc, in_maps, core_ids=list(range(NCORES)))
    LAST_RESULTS = res
    outs = [np.asarray(r["out"], dtype=np.float32) for r in res.results]
    full = np.empty((B, S, H), dtype=np.float32)
    for b in range(B):
        acc = outs[b * GROUPS]
        for g in range(1, GROUPS):
            acc = acc + outs[b * GROUPS + g]
        full[b] = acc
    return full


# revision 11
# speedup vs baseline: 1.2311x; 1.2311x over previous
"""Trainium2 Bass kernel for causal GQA attention (nn_CausalAttention).

Full-input contract: kernel(**inputs) takes the complete unsharded inputs and
returns the full [B, S, H] output. Internally shards across 8 NeuronCores as
(batch b in {0,1}) x (head-group g in {0..3}); each core computes 8 query heads
/ 2 KV heads for one batch and a row-parallel partial o_proj; the host sums the
4 partials per batch.

v2 design notes (vs the 310us baseline):
  - RoPE rotate_half is done with cross-partition-offset DVE adds (operand
    base partition != dest base partition) against a host-pre-shifted sin
    table; no SBUF->SBUF DMAs, no ACT staging copy (DVE reads PSUM directly).
  - Head-pair packing: a scores tile is (1 key tile) x (2 heads) row-packed on
    PE tiles (0,0)/(64,0); qT stores head 2m on partitions 0-63 and head 2m+1
    on 64-127 of slot m, so no qT duplication is needed. kT keeps both kv
    heads duplicated across halves (one DVE copy per chunk).
  - exp runs once per (slot, key tile) over [128, 2, span] (both heads share
    the same causal narrowing), halving ACT instruction count on diag tiles;
    the mask multiply is one broadcast TT per diag tile.
  - hidT / weights are host-relayouted to [128, k, .] so all loads are a few
    big DMAs (4 per hid chunk, 8 preload) instead of ~120 row-tile DMAs.
  - output DRAM tensor is bf16, staged per 128-row block and stored with one
    DMA per block (4/chunk); the host upcasts and sums partials in f32.
  - 1/l normalization multiplies ps_o[0:64] by ps_o[64:128] (the K=1 matmul
    broadcast) directly from PSUM - no bc staging copy.
"""

import math
import sys

import numpy as np

try:
    import concourse.bass as _probe  # noqa: F401
except ImportError:
    sys.path.insert(0, "/opt/trn_rl_repo")

import ml_dtypes

BF16 = ml_dtypes.bfloat16

# problem config (hardcoded per contract)
B, S, H = 2, 2048, 2048
NUM_HEADS, NUM_KV_HEADS, D = 32, 8, 64
NCORES = 8
GROUPS = 4                    # head-groups = cores per batch
QH = NUM_HEADS // GROUPS      # 8 q heads per core
KVH = NUM_KV_HEADS // GROUPS  # 2 kv heads per core
DQ = QH * D                   # 512
DKV = KVH * D                 # 128
KT = H // 128                 # 16 contraction tiles over hidden dim

P = 128
HD = D // 2
EXP_BIAS = -4.0

LAST_RESULTS = None
_NC_CACHE = {}


def make_cosr(cos_b, dtype=BF16):
    """cos tiled to [128, S]: row p = 64a + q holds cos[t, q]."""
    return np.ascontiguousarray(np.tile(cos_b.T, (2, 1))).astype(dtype)


def make_sinr(sin_b, dtype=BF16):
    """sin tiled to [128, S] with rotate_half's sign folded in: row p holds
    sign(partner(p)) * sin[t, d(p)] so that the shifted-output product
    prod_sh[p^32] = raw[p] * sinr[p] lands with the right sign."""
    sgn = np.where(np.arange(P) % D < HD, -1.0, 1.0).astype(np.float32)
    # sign belongs to the DESTINATION row p^32; fold sgn[p^32] into row p
    sgn_pre = sgn.reshape(2, 2, HD)[:, ::-1].reshape(P)
    return np.ascontiguousarray(
        np.tile(sin_b.T, (2, 1)) * sgn_pre[:, None]
    ).astype(dtype)


def make_diag_masks(tqc, dtype=np.float32):
    """masks[i, m, j] = 1 if key-row i of diag tile m is visible to query j."""
    ndiag = tqc // P
    masks = np.zeros((P, ndiag, tqc), dtype=np.float32)
    i = np.arange(P)[:, None]
    j = np.arange(tqc)[None, :]
    for m in range(ndiag):
        masks[:, m, :] = (i + P * m <= j).astype(np.float32)
    return masks.astype(dtype)


def build_attention_nc(S=S, H=H, TQC=512, n_repeat=1):
    """Build the single-core SPMD Bass program. TQC = query-chunk width."""
    import concourse.bass as bass  # noqa: F401
    import concourse.mybir as mybir
    import concourse.tile as tile
    from concourse import bacc
    from contextlib import ExitStack

    bf = mybir.dt.bfloat16
    f32 = mybir.dt.float32
    NTQ = S // TQC       # query chunks
    NTK = S // P         # key tiles
    GRP = TQC // P       # tiles per chunk == number of diag masks == slots
    NQO = DQ // P        # q slots (4); also yT column blocks
    NHO = H // TQC       # o_proj output chunks (4)
    scale = 1.0 / math.sqrt(D)

    nc = bacc.Bacc()
    hidT = nc.declare_dram_parameter("hidT", [P, KT, S], bf, isOutput=False)
    wqT = nc.declare_dram_parameter("wqT", [P, KT, DQ], bf, isOutput=False)
    wkT = nc.declare_dram_parameter("wkT", [P, KT, DKV], bf, isOutput=False)
    wvT = nc.declare_dram_parameter("wvT", [P, KT, DKV], bf, isOutput=False)
    woT = nc.declare_dram_parameter("woT", [P, NQO, H], bf, isOutput=False)
    cosr = nc.declare_dram_parameter("cosr", [P, S], bf, isOutput=False)
    sinr = nc.declare_dram_parameter("sinr", [P, S], bf, isOutput=False)
    masks = nc.declare_dram_parameter("masks", [P, GRP, TQC], bf, isOutput=False)
    out = nc.declare_dram_parameter("out", [S, H], bf, isOutput=True)

    MUL = mybir.AluOpType.mult
    ADD = mybir.AluOpType.add
    EXP = mybir.ActivationFunctionType.Exp

    with ExitStack() as ctx:
        tc = ctx.enter_context(tile.TileContext(nc))
        const = ctx.enter_context(tc.tile_pool(name="const", bufs=1))
        hidp = ctx.enter_context(tc.tile_pool(name="hidp", bufs=2))
        work = ctx.enter_context(tc.tile_pool(name="work", bufs=4))
        stage = ctx.enter_context(tc.tile_pool(name="stage", bufs=2))
        ppool = ctx.enter_context(tc.tile_pool(name="psmall", bufs=2, space="PSUM"))
        pbig = ctx.enter_context(tc.tile_pool(name="pbig", bufs=2, space="PSUM"))
        po = ctx.enter_context(tc.tile_pool(name="po", bufs=1, space="PSUM"))

        wq_sb = const.tile([P, KT, DQ], bf)
        wk_sb = const.tile([P, KT, DKV], bf)
        wv_sb = const.tile([P, KT, DKV], bf)
        wo_sb = const.tile([P, NQO, H], bf)
        cos_sb = const.tile([P, S], bf)
        sin_sb = const.tile([P, S], bf)
        mask_sb = const.tile([P, GRP, TQC], bf)
        # qT: slot m holds head 2m on partitions 0-63 and head 2m+1 on 64-127.
        # kT: kv heads on partitions 0-63, duplicated to 64-127 for row-packed
        # QK pairs (both heads of a pair read the same kv head).
        qT_sb = const.tile([P, NQO, S], bf)
        kT_sb = const.tile([P, KVH, S], bf)
        vA_sb = [const.tile([P, NTK, 65], bf, name=f"vA{k}") for k in range(KVH)]
        yT_sb = const.tile([P, NQO, S], bf)

        # --- preload (big DMAs; gpsimd/SWDGE for weights so the sync queue
        # stays free for hid chunks and output stores) ---
        nc.gpsimd.dma_start(wk_sb[:], wkT[:])
        nc.gpsimd.dma_start(wv_sb[:], wvT[:])
        nc.gpsimd.dma_start(cos_sb[:], cosr[:])
        nc.gpsimd.dma_start(sin_sb[:], sinr[:])
        nc.gpsimd.dma_start(mask_sb[:], masks[:])
        for f in range(NQO):
            nc.gpsimd.dma_start(wo_sb[:, f, :], woT[:, f, :])
        for k in range(KVH):
            nc.vector.memset(vA_sb[k][:, :, 64:65], 1.0)
        exp_bias_sb = const.tile([P, 1], f32)
        nc.vector.memset(exp_bias_sb[:], EXP_BIAS)
        ones_col = const.tile([1, 64], bf)
        nc.vector.memset(ones_col[:], 1.0)

        hid_chs = {}

        def emit_hid_load(c):
            tq = slice(c * TQC, (c + 1) * TQC)
            hid_ch = hidp.tile([P, KT, TQC], bf, tag="hid_ch")
            for g4 in range(0, KT, 4):
                nc.sync.dma_start(
                    hid_ch[:, g4:g4 + 4, :], hidT[:, g4:g4 + 4, tq]
                )
            hid_chs[c] = hid_ch

        def rope_project(ps, dests, tq_sl):
            """ps: [P, TQC] PSUM with 2 heads of projected values (transposed).
            dests: two [64, TQC] APs (one per 64-row head block of ps).
            rotate_half is a partition-shifted OUTPUT on the sin product (the
            BIR verifier requires equal input base partitions but lets the
            output land elsewhere); the rotate sign is folded into sinr."""
            raw = work.tile([P, TQC], bf, tag="rope_raw")
            nc.scalar.copy(raw, ps)
            t_cos = work.tile([P, TQC], bf, tag="rope_cos")
            nc.vector.tensor_tensor(t_cos, raw, cos_sb[:, tq_sl], MUL)
            prod = work.tile([P, TQC], bf, tag="rope_sin")
            for o in (0, D):
                nc.vector.tensor_tensor(
                    prod[o:o + HD, :],
                    raw[o + HD:o + D, :],
                    sin_sb[o + HD:o + D, tq_sl],
                    MUL,
                )
                nc.vector.tensor_tensor(
                    prod[o + HD:o + D, :],
                    raw[o:o + HD, :],
                    sin_sb[o:o + HD, tq_sl],
                    MUL,
                )
            for hb, dest in enumerate(dests):
                o = hb * D
                nc.vector.tensor_tensor(
                    dest, t_cos[o:o + D, :], prod[o:o + D, :], ADD
                )

        def emit_k_proj(c):
            tq = slice(c * TQC, (c + 1) * TQC)
            ps_k = ppool.tile([P, TQC], f32, tag="ps_proj")
            for ki in range(KT):
                nc.tensor.matmul(
                    ps_k,
                    wk_sb[:, ki, :],
                    hid_chs[c][:, ki, :],
                    start=(ki == 0),
                    stop=(ki == KT - 1),
                )
            rope_project(ps_k, [kT_sb[0:D, 0, tq], kT_sb[0:D, 1, tq]], tq)
            nc.vector.tensor_copy(kT_sb[D:2 * D, :, tq], kT_sb[0:D, :, tq])

        def emit_v_proj(c, tt):
            tl = tt - c * GRP
            ps_v = ppool.tile([P, TQC], f32, tag="ps_proj")
            for ki in range(KT):
                nc.tensor.matmul(
                    ps_v[:, 0:DKV],
                    hid_chs[c][:, ki, tl * P:(tl + 1) * P],
                    wv_sb[:, ki, :],
                    start=(ki == 0),
                    stop=(ki == KT - 1),
                )
            for k in range(KVH):
                nc.vector.tensor_copy(
                    vA_sb[k][:, tt, 0:64], ps_v[:, k * D:(k + 1) * D]
                )

        def emit_q_proj(c, oo):
            tq = slice(c * TQC, (c + 1) * TQC)
            ps_q = ppool.tile([P, TQC], f32, tag="ps_proj")
            for ki in range(KT):
                nc.tensor.matmul(
                    ps_q,
                    wq_sb[:, ki, oo * P:(oo + 1) * P],
                    hid_chs[c][:, ki, :],
                    start=(ki == 0),
                    stop=(ki == KT - 1),
                )
            rope_project(
                ps_q, [qT_sb[0:D, oo, tq], qT_sb[D:2 * D, oo, tq]], tq
            )

        ostage = {}

        def emit_oproj_unit(c, tt, oc):
            ps_out = ppool.tile([P, TQC], f32, tag="ps_proj")
            for f in range(NQO):
                nc.tensor.matmul(
                    ps_out,
                    yT_sb[:, f, tt * P:(tt + 1) * P],
                    wo_sb[:, f, oc * TQC:(oc + 1) * TQC],
                    start=(f == 0),
                    stop=(f == NQO - 1),
                )
            if oc == 0:
                ostage[tt] = stage.tile(
                    [P, NHO, TQC], bf, tag="ostage", name="ostage"
                )
            st = ostage[tt]
            nc.vector.tensor_copy(st[:, oc, :], ps_out[:])
            if oc == NHO - 1:
                nc.sync.dma_start(out[tt * P:(tt + 1) * P, :], st[:])
                del ostage[tt]

        def run_main_loop():
            for c in range(NTQ):
                _run_chunk(c)
            for tt in range((NTQ - 1) * GRP, NTQ * GRP):
                for oc in range(NHO):
                    emit_oproj_unit(NTQ - 1, tt, oc)

        def _run_chunk(c):
            tq_sl = slice(c * TQC, (c + 1) * TQC)
            if c + 1 < NTQ:
                emit_hid_load(c + 1)
            fillers = []
            if c + 1 < NTQ:
                fillers.append(lambda cc=c + 1: emit_k_proj(cc))
                for tt in range((c + 1) * GRP, (c + 2) * GRP):
                    fillers.append(lambda cc=c + 1, t=tt: emit_v_proj(cc, t))
                for oo in range(NQO):
                    fillers.append(lambda cc=c + 1, o=oo: emit_q_proj(cc, o))
            if c >= 1:
                for tt in range((c - 1) * GRP, c * GRP):
                    for oc in range(NHO):
                        fillers.append(
                            lambda cc=c - 1, t=tt, o=oc: emit_oproj_unit(cc, t, o)
                        )
            popped = 0

            ntk = (c + 1) * GRP
            for m in range(GRP):  # head-pair slots
                kv = m // 2
                ps_o0 = po.tile([P, TQC], f32, tag="ps_o0")
                ps_o1 = po.tile([P, TQC], f32, tag="ps_o1")
                pos = (ps_o0, ps_o1)
                for t in range(ntk):
                    md = t - (ntk - GRP)
                    lo = md * P if md >= 0 else 0
                    ps_s = pbig.tile([P, 2, TQC], f32, tag="ps_s")
                    for j in range(2):
                        pb = D * j
                        nc.tensor.matmul(
                            ps_s[:, j, lo:],
                            kT_sb[pb:pb + D, kv, t * P:(t + 1) * P],
                            qT_sb[pb:pb + D, m, c * TQC + lo:(c + 1) * TQC],
                            start=True,
                            stop=True,
                        )
                    p_sb = work.tile([P, 2, TQC], bf, tag="p_sb")
                    nc.scalar.activation(
                        p_sb[:, :, lo:],
                        ps_s[:, :, lo:],
                        EXP,
                        bias=exp_bias_sb[:],
                        scale=scale,
                    )
                    if md >= 0:
                        # only the diag 128-column block straddles the causal
                        # boundary; columns >= lo+P are fully visible
                        hi = lo + P
                        mop = mask_sb[:, md, lo:hi].unsqueeze(1).broadcast_to(
                            (P, 2, P)
                        )
                        nc.vector.tensor_tensor(
                            p_sb[:, :, lo:hi], p_sb[:, :, lo:hi], mop, MUL
                        )
                    for j in range(2):
                        nc.tensor.matmul(
                            pos[j][0:65, lo:],
                            vA_sb[kv][:, t, :],
                            p_sb[:, j, lo:],
                            start=(t == 0),
                            stop=(t == ntk - 1),
                        )
                    # smooth PE filler across the tile loop, not just at
                    # slot boundaries (the exp-bound inner loop starves PE)
                    want = len(fillers) * (m * ntk + t + 1) // (GRP * ntk)
                    while popped < want:
                        fillers[popped]()
                        popped += 1
                for j in range(2):
                    ps_o = pos[j]
                    rec = work.tile([1, TQC], bf, tag="rec")
                    with nc.allow_low_precision(
                        reason="1/l broadcast via bf16 matmul; ~0.2% uniform scale noise"
                    ):
                        nc.vector.reciprocal(rec, ps_o[64:65, :])
                    nc.tensor.matmul(
                        ps_o[64:128, :], ones_col[:], rec[:], start=True, stop=True
                    )
                    # DVE may read only one operand from PSUM: stage the
                    # broadcast 1/l rows through SBUF
                    bc = work.tile([64, TQC], f32, tag="bc")
                    nc.vector.tensor_copy(bc[:], ps_o[64:128, :])
                    nc.vector.tensor_tensor(
                        yT_sb[D * j:D * (j + 1), m, tq_sl],
                        ps_o[0:64, :],
                        bc[:],
                        MUL,
                    )
                want = len(fillers) * (m + 1) // GRP
                while popped < want:
                    fillers[popped]()
                    popped += 1

        first = True
        for _rep in range(n_repeat):
            if first:
                emit_hid_load(0)
                # wq after hid0 on the sync queue: k/v proj need hid first
                for g4 in range(0, KT, 4):
                    nc.sync.dma_start(
                        wq_sb[:, g4:g4 + 4, :], wqT[:, g4:g4 + 4, :]
                    )
                first = False
            else:
                emit_hid_load(0)
            emit_k_proj(0)
            for tt in range(GRP):
                emit_v_proj(0, tt)
            for oo in range(NQO):
                emit_q_proj(0, oo)
            run_main_loop()

    nc.compile()
    return nc


def _get_nc():
    key = (S, H)
    if key not in _NC_CACHE:
        _NC_CACHE[key] = build_attention_nc()
    return _NC_CACHE[key]


def _prep_core_inputs(hidden_states, cos, sin, Wq, Wk, Wv, Wo):
    """Build the 8 per-core input maps (core index = b * GROUPS + g)."""
    TQC = 512
    masks_np = make_diag_masks(TQC, BF16)

    def relay(mat, k):
        """[k*128, n] -> [128, k, n] so row p, slice ki holds mat[ki*128+p]."""
        n = mat.shape[1]
        return np.ascontiguousarray(
            mat.reshape(k, P, n).transpose(1, 0, 2)
        ).astype(BF16)

    in_maps = []
    per_batch = {}
    for b in range(B):
        hidT = relay(np.ascontiguousarray(hidden_states[b].T), KT)
        cosr = make_cosr(cos[b])
        sinr = make_sinr(sin[b])
        per_batch[b] = (hidT, cosr, sinr)
    wq_g = [relay(np.ascontiguousarray(Wq[DQ * g:DQ * (g + 1), :].T), KT)
            for g in range(GROUPS)]
    wk_g = [relay(np.ascontiguousarray(Wk[DKV * g:DKV * (g + 1), :].T), KT)
            for g in range(GROUPS)]
    wv_g = [relay(np.ascontiguousarray(Wv[DKV * g:DKV * (g + 1), :].T), KT)
            for g in range(GROUPS)]
    wo_g = [relay(np.ascontiguousarray(Wo[:, DQ * g:DQ * (g + 1)].T), DQ // P)
            for g in range(GROUPS)]
    for b in range(B):
        hidT, cosr, sinr = per_batch[b]
        for g in range(GROUPS):
            in_maps.append({
                "hidT": hidT,
                "wqT": wq_g[g],
                "wkT": wk_g[g],
                "wvT": wv_g[g],
                "woT": wo_g[g],
                "cosr": cosr,
                "sinr": sinr,
                "masks": masks_np,
            })
    return in_maps


def kernel(hidden_states, cos, sin, Wq, Wk, Wv, Wo):
    global LAST_RESULTS
    from concourse.bass_utils import run_bass_kernel_spmd

    hidden_states = np.asarray(hidden_states, dtype=np.float32)
    cos = np.asarray(cos, dtype=np.float32)
    sin = np.asarray(sin, dtype=np.float32)
    Wq = np.asarray(Wq, dtype=np.float32)
    Wk = np.asarray(Wk, dtype=np.float32)
    Wv = np.asarray(Wv, dtype=np.float32)
    Wo = np.asarray(Wo, dtype=np.float32)
    assert hidden_states.shape == (B, S, H)

    nc = _get_nc()
    in_maps = _prep_core_inputs(hidden_states, cos, sin, Wq, Wk, Wv, Wo)
    res = run_bass_kernel_spmd(nc, in_maps, core_ids=list(range(NCORES)))
    LAST_RESULTS = res
    outs = [np.asarray(r["out"], dtype=np.float32) for r in res.results]
    full = np.empty((B, S, H), dtype=np.float32)
    for b in range(B):
        acc = outs[b * GROUPS]
        for g in range(1, GROUPS):
            acc = acc + outs[b * GROUPS + g]
        full[b] = acc
    return full
